# revision 32
# baseline (speedup 1.0000x reference)
"""Trainium2 Bass kernel for nn_BERTClassifier (batch-mixing attention BERT).

Only position 0 of the sequence reaches the output (attention mixes the batch
within a position; everything else is positionwise), so the real work is a
[32, 768] activation through 4 layers. Architecture (HW exec ~211us vs 388us
baseline):

- Attention weights REPLICATED on all 8 cores (attention is small and
  recomputing it beats paying collective latency). The FFN of layers 0-1 is
  fully REPLICATED too, so those layers need no collective at all and run
  entirely under the ~110us ncfw startup (barrier + first-op ramp); layers
  2-3 shard the FFN 8-way and each needs exactly ONE ncfw AllReduce of the
  o2 partials (fp16, 48KB, ~10us warm).
- A tiny fire-and-forget warmup AllReduce is issued at t~0: it starts the
  ncfw init under the weight stream, absorbs the expensive first-op cost,
  and (critically) makes the runtime launch all 8 cores in lockstep -- a
  collective-free NEFF gets per-core launch skew of milliseconds.
- All weights are cast to fp16 AND pre-arranged into the exact SBUF tile
  layout on the HOST, so device DMAs are plain contiguous HWDGE transfers
  (~350GB/s observed; no SWDGE casting). Chain-critical small DMAs (AR
  bounces, x0) ride the scalar HWDGE ring so they never queue behind bulk
  weight DMA on the sync ring.
- Residual stream kept feature-major [128, 6, 32] fp16 (vector ops at packed
  rate, no fp16 staging copies); LayerNorm stats via PE ones-vector matmuls
  with 1/E folded into the ones constant; softmax skips max-subtraction
  (scores*scale is small); V's projection matmuls are emitted after the
  scores so the PE works under the softmax's vector/scalar ops.
- HAM keepalive: dummy fp32 matmuls pinned into each AllReduce gap keep the
  PE activity monitor busy so it stays at 2.4GHz instead of dropping to
  1.2GHz after >3.4us idle.
- fp16 matmuls with fp32 PSUM accumulation throughout; rel err ~1.8e-3.

Self-contained: shapes hardcoded, no sibling imports.
"""
import os
import sys
import types

import numpy as np

# If BASS_TRACE is set but the axon NTFF hook module is absent, bass_utils
# would crash importing antenv.axon_hooks. Provide a null hook so tracing
# degrades to a warning instead. (test.py installs the real hook first.)
try:
    from antenv import axon_hooks as _ah  # noqa: F401
except ImportError:
    try:
        import antenv as _antenv
        _mod = types.ModuleType("antenv.axon_hooks")
        _mod.get_axon_ntff_profile_hook = lambda: None
        _mod.set_axon_ntff_profile_hook = lambda h: None
        _antenv.axon_hooks = _mod
        sys.modules["antenv.axon_hooks"] = _mod
    except Exception:
        pass

import concourse.bass as bass
import concourse.bacc as bacc
import concourse.mybir as mybir
import concourse.tile as tile
from concourse import masks
from concourse.bass_utils import run_bass_kernel_spmd

F32 = mybir.dt.float32
F16 = mybir.dt.float16
AX = mybir.AxisListType
ALU = mybir.AluOpType
ACT_F = mybir.ActivationFunctionType

V, E, F, L, S, B, C = 30522, 768, 3072, 4, 512, 32, 2
NC = 8            # cores
KC = E // 128     # 6 contraction chunks of 128
MQ = (3 * E) // 128   # 18 output blocks for merged QKV
NSH = 8           # FFN shard ways (layers >= 1; layer 0's FFN is replicated
                  # so it needs no collective and can finish during the ~75us
                  # ncfw startup barrier)
FSH = F // NSH    # 384 ffn shard
MH = FSH // 128   # 3 output blocks for FFN1 shard
def _mh(l):
    return F // 128 if l < 3 else MH
SCALE = 1.0 / float(np.sqrt(E))
EPS = 1e-5

_CACHE = {}
LAST_RESULT = None  # BassKernelResults of the most recent run (for test.py)


def _declare(nc, use_bias, use_affine):
    h = {}
    h["x0T"] = nc.dram_tensor("x0T", [128, KC * B], F16, kind="ExternalInput")
    for l in range(L):
        h[f"wqkv{l}"] = nc.dram_tensor(f"wqkv{l}", [128, MQ * KC * 128], F16, kind="ExternalInput")
        h[f"wo{l}"] = nc.dram_tensor(f"wo{l}", [128, KC * KC * 128], F16, kind="ExternalInput")
        h[f"w1{l}"] = nc.dram_tensor(f"w1{l}", [128, KC * _mh(l) * 128], F16, kind="ExternalInput")
        h[f"w2{l}"] = nc.dram_tensor(f"w2{l}", [128, _mh(l) * KC * 128], F16, kind="ExternalInput")
        if use_bias:
            pass
        if use_bias:
            h[f"bqkv{l}"] = nc.dram_tensor(f"bqkv{l}", [128, MQ], F32, kind="ExternalInput")
            h[f"bo{l}"] = nc.dram_tensor(f"bo{l}", [128, KC], F32, kind="ExternalInput")
            h[f"bf1{l}"] = nc.dram_tensor(f"bf1{l}", [128, _mh(l)], F32, kind="ExternalInput")
            h[f"bf2{l}"] = nc.dram_tensor(f"bf2{l}", [128, KC], F32, kind="ExternalInput")
        if use_affine:
            h[f"g1{l}"] = nc.dram_tensor(f"g1{l}", [128, KC], F32, kind="ExternalInput")
            h[f"be1{l}"] = nc.dram_tensor(f"be1{l}", [128, KC], F32, kind="ExternalInput")
            h[f"g2{l}"] = nc.dram_tensor(f"g2{l}", [128, KC], F32, kind="ExternalInput")
            h[f"be2{l}"] = nc.dram_tensor(f"be2{l}", [128, KC], F32, kind="ExternalInput")
    h["wc"] = nc.dram_tensor("wc", [128, KC * C], F16, kind="ExternalInput")
    if use_bias:
        h["bc"] = nc.dram_tensor("bc", [B, C], F32, kind="ExternalInput")
    h["out"] = nc.dram_tensor("out", [B, C], F32, kind="ExternalOutput")
    return h


def _emit(tc, h, use_bias, use_affine):
    nc = tc.nc
    ctxs = []
    rx_waits = []  # (reduce BassInstruction, threshold) — waits attached post-schedule

    def pool(*a, **k):
        p = tc.alloc_tile_pool(*a, **k)
        ctxs.append(p)
        return p

    const = pool(name="const", bufs=1)
    wp = pool(name="wts", bufs=3)
    ab = pool(name="act", bufs=2)
    ps = pool(name="ps", bufs=2, space="PSUM")
    dr = pool(name="dram", bufs=2, space="DRAM")

    groups = [list(range(NC))]
    rx_sem = None

    wu_sb = const.tile([1, 16], F32)
    nc.vector.memset(wu_sb[:], 0.0)
    wu_i = dr.tile([16], F32, tag="wui")
    wu_o = dr.tile([16], F32, addr_space="Shared", tag="wuo")
    nc.scalar.dma_start(wu_i[:].rearrange("(o x) -> o x", o=1), wu_sb[:])
    nc.gpsimd.collective_compute(
        "AllReduce", ALU.add, replica_groups=groups,
        ins=[wu_i.opt()], outs=[wu_o.opt()],
    )

    # ---- constants
    ones_col = const.tile([128, 1], F32)
    nc.vector.memset(ones_col[:], 1.0 / E)
    ones_col16 = const.tile([128, 1], F16)
    nc.vector.memset(ones_col16[:], 1.0 / E)
    ones_row = const.tile([1, 128], F32)
    nc.vector.memset(ones_row[:], 1.0)
    eps_sb = const.tile([1, 1], F32)
    nc.vector.memset(eps_sb[:], EPS)
    ident128 = const.tile([128, 128], F16)
    masks.make_identity(nc, ident128[:])
    warm_src = const.tile([128, 512], F32)
    nc.vector.memset(warm_src[:], 1.0)

    # ---- x0 (host-combined tok+pos embedding, feature-major)
    xT = ab.tile([128, KC, B], F16, tag="xt")
    nc.scalar.dma_start(xT[:], h["x0T"].ap().rearrange("p (k b) -> p k b", k=KC))

    def load_vec(name):
        # [128, n] f32 per-feature column vectors (bias/affine)
        t = wp.tile([128, h[name].shape[1]], F32, tag=name[:3])
        nc.sync.dma_start(t[:], h[name].ap())
        return t

    def layernorm(yT, g=None, be=None):
        sq = ab.tile([128, KC, B], F32, tag="sq")
        nc.vector.tensor_tensor(sq[:], yT[:], yT[:], op=ALU.mult)
        s1 = ps.tile([1, B], F32, tag="st")
        s2 = ps.tile([1, B], F32, tag="st")
        for k in range(KC):
            nc.tensor.matmul(s1[:], ones_col16[:], yT[:, k, :], start=(k == 0), stop=(k == KC - 1))
        for k in range(KC):
            nc.tensor.matmul(s2[:], ones_col[:], sq[:, k, :], start=(k == 0), stop=(k == KC - 1))
        # s1 = mean, s2 = E[x^2] (ones_col holds 1/E)
        mean = ab.tile([1, B], F32, tag="mean")
        nc.vector.tensor_copy(mean[:], s1[:])
        msq = ab.tile([1, B], F32, tag="msq")
        nc.scalar.activation(msq[:], mean[:], ACT_F.Square)
        var = ab.tile([1, B], F32, tag="var")
        nc.vector.tensor_tensor(var[:], s2[:], msq[:], op=ALU.subtract)
        sd = ab.tile([1, B], F32, tag="sd")
        nc.scalar.activation(sd[:], var[:], ACT_F.Sqrt, bias=eps_sb[:])
        rstd = ab.tile([1, B], F32, tag="rstd")
        nc.vector.reciprocal(rstd[:], sd[:])
        mu_b = ps.tile([128, B], F32, tag="st")
        nc.tensor.matmul(mu_b[:], ones_row[:], mean[:], start=True, stop=True)
        rs_b = ps.tile([128, B], F32, tag="st")
        nc.tensor.matmul(rs_b[:], ones_row[:], rstd[:], start=True, stop=True)
        xn = ab.tile([128, KC, B], F16, tag="xn")
        tmp = ab.tile([128, KC, B], F16, tag="lntmp")
        mu_bb = mu_b[:].rearrange("p (o b) -> p o b", o=1).broadcast_to([128, KC, B])
        rs_bb = rs_b[:].rearrange("p (o b) -> p o b", o=1).broadcast_to([128, KC, B])
        nc.vector.tensor_tensor(tmp[:], yT[:], mu_bb, op=ALU.subtract)
        nc.vector.tensor_tensor(xn[:], tmp[:], rs_bb, op=ALU.mult)
        if g is not None:
            for k in range(KC):
                if be is not None:
                    nc.vector.tensor_scalar(
                        xn[:, k, :], xn[:, k, :], g[:, k:k + 1], be[:, k:k + 1],
                        ALU.mult, ALU.add)
                else:
                    nc.vector.tensor_scalar_mul(xn[:, k, :], xn[:, k, :], g[:, k:k + 1])
        elif be is not None:
            for k in range(KC):
                nc.vector.tensor_scalar_add(xn[:, k, :], xn[:, k, :], be[:, k:k + 1])
        return xn

    for l in range(L):
        # ---- weight loads: plain fp16 HWDGE streams, host-prearranged layout
        wqkv_t = wp.tile([128, MQ * KC, 128], F16, tag="wqkv", bufs=1)
        nc.sync.dma_start(wqkv_t[:], h[f"wqkv{l}"].ap().rearrange("p (x f) -> p x f", f=128))
        wo_t = wp.tile([128, KC * KC, 128], F16, tag="wo", bufs=1)
        nc.sync.dma_start(wo_t[:], h[f"wo{l}"].ap().rearrange("p (x f) -> p x f", f=128))
        mh = _mh(l)
        w1_t = wp.tile([128, KC * mh, 128], F16,
                       tag="w1big" if l < 3 else f"w1_{l}", bufs=1)
        nc.sync.dma_start(w1_t[:], h[f"w1{l}"].ap().rearrange("p (x f) -> p x f", f=128))
        w2_t = wp.tile([128, mh * KC, 128], F16,
                       tag="w2big" if l < 3 else f"w2_{l}", bufs=1)
        nc.sync.dma_start(w2_t[:], h[f"w2{l}"].ap().rearrange("p (x f) -> p x f", f=128))
        bqkv = load_vec(f"bqkv{l}") if use_bias else None
        bo = load_vec(f"bo{l}") if use_bias else None
        bf1 = load_vec(f"bf1{l}") if use_bias else None
        bf2 = load_vec(f"bf2{l}") if use_bias else None
        g1 = load_vec(f"g1{l}") if use_affine else None
        be1 = load_vec(f"be1{l}") if use_affine else None
        g2 = load_vec(f"g2{l}") if use_affine else None
        be2 = load_vec(f"be2{l}") if use_affine else None

        # ---- merged QKV^T, feature-major [128, 18, 32]
        qkv_ps = ps.tile([128, MQ, B], F32, tag="qkv", bufs=1)
        for m in range(2 * KC):
            for k in range(KC):
                nc.tensor.matmul(qkv_ps[:, m, :], wqkv_t[:, k * MQ + m, :], xT[:, k, :],
                                 start=(k == 0), stop=(k == KC - 1))
        qkvT = ab.tile([128, MQ, B], F16, tag="qkvT")
        if use_bias:
            for m in range(2 * KC):
                nc.vector.tensor_scalar_add(qkvT[:, m, :], qkv_ps[:, m, :], bqkv[:, m:m + 1])
        else:
            nc.vector.tensor_copy(qkvT[:, 0:2 * KC, :], qkv_ps[:, 0:2 * KC, :])

        # ---- scores + softmax (token-major [32, 32]); V's matmuls run on the
        # PE underneath the vector/scalar softmax work
        sc_ps = ps.tile([B, B], F32, tag="sc", bufs=1)
        for k in range(KC):
            nc.tensor.matmul(sc_ps[:], qkvT[:, k, :], qkvT[:, KC + k, :],
                             start=(k == 0), stop=(k == KC - 1))
        for m in range(2 * KC, MQ):
            for k in range(KC):
                nc.tensor.matmul(qkv_ps[:, m, :], wqkv_t[:, k * MQ + m, :], xT[:, k, :],
                                 start=(k == 0), stop=(k == KC - 1))
        if use_bias:
            for m in range(2 * KC, MQ):
                nc.vector.tensor_scalar_add(qkvT[:, m, :], qkv_ps[:, m, :], bqkv[:, m:m + 1])
        else:
            nc.vector.tensor_copy(qkvT[:, 2 * KC:MQ, :], qkv_ps[:, 2 * KC:MQ, :])
        attn = ab.tile([B, B], F32, tag="attn")
        rsum = ab.tile([B, 1], F32, tag="rsum")
        nc.scalar.activation(attn[:], sc_ps[:], ACT_F.Exp, scale=SCALE,
                             accum_out=rsum[:])
        rinv = ab.tile([B, 1], F32, tag="rinv")
        nc.vector.reciprocal(rinv[:], rsum[:])
        attn_n = ab.tile([B, B], F16, tag="attn_n")
        nc.vector.tensor_scalar_mul(attn_n[:], attn[:], rinv[:])
        attnT = ab.tile([B, B], F16, tag="attnT")
        nc.vector.transpose(attnT[:], attn_n[:])

        # ---- V token-major via PE transposes, then ao^T, then o^T = Wo^T ao^T
        v_ps = ps.tile([B, KC, 128], F16, tag="mm")
        for k in range(KC):
            nc.tensor.transpose(v_ps[:, k, :], qkvT[:, 2 * KC + k, :], ident128[:])
        v_sb = ab.tile([B, KC, 128], F16, tag="vsb")
        nc.vector.tensor_copy(v_sb[:], v_ps[:])
        ao_ps = ps.tile([128, KC, B], F32, tag="mm")
        for m in range(KC):
            nc.tensor.matmul(ao_ps[:, m, :], v_sb[:, m, :], attnT[:], start=True, stop=True)
        aoT = ab.tile([128, KC, B], F16, tag="aoT")
        nc.vector.tensor_copy(aoT[:], ao_ps[:])
        oT_ps = ps.tile([128, KC, B], F32, tag="mm")
        for m in range(KC):
            for k in range(KC):
                nc.tensor.matmul(oT_ps[:, m, :], wo_t[:, k * KC + m, :], aoT[:, k, :],
                                 start=(k == 0), stop=(k == KC - 1))

        # ---- residual + LN1
        y1 = ab.tile([128, KC, B], F16, tag="y1")
        y1_add = nc.vector.tensor_tensor(y1[:], xT[:], oT_ps[:], op=ALU.add)
        if use_bias:
            for k in range(KC):
                nc.vector.tensor_scalar_add(y1[:, k, :], y1[:, k, :], bo[:, k:k + 1])
        x1n = layernorm(y1, g1, be1)

        # ---- FFN1 shard: h^T = relu(W1_c^T x1n)  [128, 3, 32]
        hT_ps = ps.tile([128, mh, B], F32, tag="mm" if l >= 3 else "qkv", bufs=2 if l >= 3 else 1)
        for m in range(mh):
            for k in range(KC):
                nc.tensor.matmul(hT_ps[:, m, :], w1_t[:, k * mh + m, :], x1n[:, k, :],
                                 start=(k == 0), stop=(k == KC - 1))
        if use_bias:
            for m in range(mh):
                nc.vector.tensor_scalar_add(hT_ps[:, m, :], hT_ps[:, m, :], bf1[:, m:m + 1])
        hT = ab.tile([128, mh, B], F16, tag=f"hT{l}", bufs=1)
        nc.vector.tensor_scalar_max(hT[:], hT_ps[:], 0.0)

        # ---- FFN2: o2^T (full for layer 0, shard partial for layers >= 1)
        o2_ps = ps.tile([128, KC, B], F32, tag="mm")
        for m in range(KC):
            for t in range(mh):
                nc.tensor.matmul(o2_ps[:, m, :], w2_t[:, t * KC + m, :], hT[:, t, :],
                                 start=(t == 0), stop=(t == mh - 1))
        if l < 3:
            y2 = ab.tile([128, KC, B], F16, tag="y2")
            y2_add = nc.vector.tensor_tensor(y2[:], x1n[:], o2_ps[:], op=ALU.add)
            if l in (1, 2):
                # ncfw keepalives: after its last op finishes, ncfw re-sleeps
                # and the next op pays a ~20us wake penalty (AR-2 measured
                # 29us after an idle gap vs ~10us back-to-back). Two thrown-
                # away collectives, semaphore-pinned into layer 1, keep it
                # busy until the first real AllReduce arrives.
                for ki, anchor in ((2 * l - 2, y1_add), (2 * l - 1, y2_add)):
                    ka_i = dr.tile([16], F32, tag=f"ka{ki}i")
                    ka_o = dr.tile([16], F32, addr_space="Shared", tag=f"ka{ki}o")
                    kd = nc.scalar.dma_start(
                        ka_i[:].rearrange("(o x) -> o x", o=1), wu_sb[:])
                    tile.add_dep_helper(kd.ins, anchor.ins, sync=True,
                                        reason="ncfw keepalive pacing")
                    nc.gpsimd.collective_compute(
                        "AllReduce", ALU.add, replica_groups=groups,
                        ins=[ka_i.opt()], outs=[ka_o.opt()],
                    )
            if use_bias:
                for k in range(KC):
                    nc.vector.tensor_scalar_add(y2[:, k, :], y2[:, k, :], bf2[:, k:k + 1])
            xT = layernorm(y2, g2, be2)
            continue
        o2s = ab.tile([128, KC, B], F16, tag=f"o2s{l}", bufs=1)
        nc.vector.tensor_copy(o2s[:], o2_ps[:])

        o2g = ab.tile([128, KC * B], F16, tag="o2g")
        if True:
            ar_i = dr.tile([128 * KC * B], F16, tag="ari")
            ar_o = dr.tile([128 * KC * B], F16, addr_space="Shared", tag="aro")
            nc.scalar.dma_start(
                ar_i[:].rearrange("(p k b) -> p k b", p=128, k=KC), o2s[:])
            cc = nc.gpsimd.collective_compute(
                "AllReduce", ALU.add, replica_groups=groups,
                ins=[ar_i.opt()], outs=[ar_o.opt()],
            )
            nc.scalar.dma_start(
                o2g[:].rearrange("p (k b) -> p k b", k=KC),
                ar_o[:].rearrange("(p k b) -> p k b", p=128, k=KC))
            # HAM keepalive: the AR leaves the PE idle past the ~3.4us HAM
            # window, dropping it to 1.2GHz for the next layer. Dummy fp32
            # matmuls (~0.85us each) pinned behind the AR trigger keep the
            # activity monitor busy; results go to a scratch PSUM bank.
            warm_ps = ps.tile([1, 512], F32, tag="sc", bufs=1)
            for _ in range(8):
                d = nc.tensor.matmul(warm_ps[:], ones_col[:], warm_src[:],
                                     start=True, stop=True)
                tile.add_dep_helper(d.ins, cc.ins, sync=False,
                                    reason="HAM keepalive pinned in AR gap")

        # ---- residual + LN2
        y2 = ab.tile([128, KC, B], F16, tag="y2")
        nc.vector.tensor_tensor(
            y2[:], x1n[:], o2g[:].rearrange("p (k b) -> p k b", k=KC), op=ALU.add)
        if use_bias:
            for k in range(KC):
                nc.vector.tensor_scalar_add(y2[:, k, :], y2[:, k, :], bf2[:, k:k + 1])
        xT = layernorm(y2, g2, be2)

    # ---- classifier
    wc_sb = wp.tile([128, KC, C], F16, tag="wc")
    nc.sync.dma_start(wc_sb[:], h["wc"].ap().rearrange("p (k n) -> p k n", k=KC))
    lg_ps = ps.tile([B, C], F32, tag="sc", bufs=1)
    for k in range(KC):
        nc.tensor.matmul(lg_ps[:], xT[:, k, :], wc_sb[:, k, :], start=(k == 0), stop=(k == KC - 1))
    lg_sb = ab.tile([B, C], F32, tag="lgs")
    if use_bias:
        bc_sb = wp.tile([B, C], F32, tag="bcs")
        nc.sync.dma_start(bc_sb[:], h["bc"].ap())
        nc.vector.tensor_tensor(lg_sb[:], lg_ps[:], bc_sb[:], op=ALU.add)
    else:
        nc.vector.tensor_copy(lg_sb[:], lg_ps[:])
    nc.scalar.dma_start(h["out"].ap(), lg_sb[:])

    for p in reversed(ctxs):
        p.release()
    return rx_waits, rx_sem


def build(use_bias, use_affine):
    key = (use_bias, use_affine)
    if key in _CACHE:
        return _CACHE[key]
    nc = bacc.Bacc("TRN2", target_bir_lowering=False, debug=False, num_devices=NC)
    h = _declare(nc, use_bias, use_affine)
    with tile.TileContext(nc) as tc:
        rx_waits, rx_sem = _emit(tc, h, use_bias, use_affine)
    # Attach the true cross-core arrival waits AFTER scheduling: the Tile
    # scheduling sim cannot model remote semaphore increments and would
    # report a false deadlock. The wait condition rides on the reduce
    # instruction itself, so it is enforced wherever it was scheduled.
    for red, thr in rx_waits:
        red.wait_op(rx_sem, thr, "sem-ge")
    nc.compile()
    _CACHE[key] = (nc, h)
    return nc, h


def _prep_w(w, nblk):
    # [rows, cols] fp32 -> [128, (rows/128)*(cols/128)*128] fp16, laid out so
    # SBUF tile slice [:, k*nblk+m, :] is the [128,128] block W[128k:, 128m:].
    rows, cols = w.shape
    k = rows // 128
    assert cols == nblk * 128
    return np.ascontiguousarray(
        w.reshape(k, 128, nblk, 128).transpose(1, 0, 2, 3).reshape(128, -1)
        .astype(np.float16))


def _prep_vec(v):
    # [n] -> [128, n/128] f32 feature-major column layout
    n = v.shape[0]
    return np.ascontiguousarray(v.reshape(n // 128, 128).T.astype(np.float32))


def make_in_maps(inputs, use_bias, use_affine):
    inp = {k: np.asarray(v, dtype=np.float32)
           if np.asarray(v).dtype not in (np.int32, np.int64) else np.asarray(v)
           for k, v in inputs.items()}
    ids = np.asarray(inputs["input_ids"])[0]
    x0 = inp["tok_emb"][ids] + inp["pos_emb"][0][None, :]       # [32, 768]
    x0T = np.ascontiguousarray(
        x0.T.reshape(KC, 128, B).transpose(1, 0, 2).reshape(128, KC * B)
        .astype(np.float16))

    # replicated tensors (same arrays shared across cores)
    rep = {"x0T": x0T}
    for l in range(L):
        wqkv = np.concatenate([inp["Wq"][l], inp["Wk"][l], inp["Wv"][l]], axis=1)
        rep[f"wqkv{l}"] = _prep_w(wqkv, MQ)
        rep[f"wo{l}"] = _prep_w(inp["Wo"][l], KC)
        if use_bias:
            bqkv = np.concatenate([inp["bq"][l], inp["bk"][l], inp["bv"][l]])
            rep[f"bqkv{l}"] = _prep_vec(bqkv)
            rep[f"bo{l}"] = _prep_vec(inp["bo"][l])
            rep[f"bf2{l}"] = _prep_vec(inp["bf2"][l])
        if use_affine:
            rep[f"g1{l}"] = _prep_vec(inp["g1"][l])
            rep[f"be1{l}"] = _prep_vec(inp["beta1"][l])
            rep[f"g2{l}"] = _prep_vec(inp["g2"][l])
            rep[f"be2{l}"] = _prep_vec(inp["beta2"][l])
    rep["wc"] = np.ascontiguousarray(
        inp["Wc"].reshape(KC, 128, C).transpose(1, 0, 2).reshape(128, KC * C)
        .astype(np.float16))
    if use_bias:
        rep["bc"] = np.ascontiguousarray(
            np.broadcast_to(inp["bc"][None, :], (B, C)).astype(np.float32))

    rep_w1f = [_prep_w(inp["W1"][l], F // 128) for l in range(3)]
    rep_w2f = [_prep_w(inp["W2"][l], KC) for l in range(3)]
    rep_bf1f = [_prep_vec(inp["bf1"][l]) for l in range(3)] if use_bias else None
    in_maps = []
    for c in range(NC):
        s = c & (NSH - 1)  # cores c and c^4 carry the same FFN shard
        m = dict(rep)
        for l in range(L):
            if l < 3:
                m[f"w1{l}"] = rep_w1f[l]
                m[f"w2{l}"] = rep_w2f[l]
                if use_bias:
                    m[f"bf1{l}"] = rep_bf1f[l]
                continue
            m[f"w1{l}"] = _prep_w(
                np.ascontiguousarray(inp["W1"][l][:, FSH * s:FSH * (s + 1)]), MH)
            m[f"w2{l}"] = _prep_w(
                np.ascontiguousarray(inp["W2"][l][FSH * s:FSH * (s + 1), :]), KC)
            if use_bias:
                m[f"bf1{l}"] = _prep_vec(inp["bf1"][l][FSH * s:FSH * (s + 1)])
        in_maps.append(m)
    return in_maps


def _flags(inputs):
    z = lambda *names: all(not np.any(np.asarray(inputs[n])) for n in names)
    use_bias = not z("bq", "bk", "bv", "bo", "bf1", "bf2", "bc")
    use_affine = not (
        z("beta1", "beta2")
        and np.all(np.asarray(inputs["g1"]) == 1.0)
        and np.all(np.asarray(inputs["g2"]) == 1.0)
    )
    return use_bias, use_affine


def kernel(**inputs) -> np.ndarray:
    global LAST_RESULT
    use_bias, use_affine = _flags(inputs)
    nc, h = build(use_bias, use_affine)
    in_maps = make_in_maps(inputs, use_bias, use_affine)
    res = run_bass_kernel_spmd(nc, in_maps, core_ids=list(range(NC)))
    LAST_RESULT = res
    return np.asarray(res.results[0]["out"])


# revision 33
# speedup vs baseline: 1.0360x; 1.0360x over previous
"""Trainium2 Bass kernel for nn_BERTClassifier (batch-mixing attention BERT).

Only position 0 of the sequence reaches the output (attention mixes the batch
within a position; everything else is positionwise), so the real work is a
[32, 768] activation through 4 layers. Architecture (HW exec ~211us vs 388us
baseline):

- Attention weights REPLICATED on all 8 cores (attention is small and
  recomputing it beats paying collective latency). The FFN of layers 0-1 is
  fully REPLICATED too, so those layers need no collective at all and run
  entirely under the ~110us ncfw startup (barrier + first-op ramp); layers
  2-3 shard the FFN 8-way and each needs exactly ONE ncfw AllReduce of the
  o2 partials (fp16, 48KB, ~10us warm).
- A tiny fire-and-forget warmup AllReduce is issued at t~0: it starts the
  ncfw init under the weight stream, absorbs the expensive first-op cost,
  and (critically) makes the runtime launch all 8 cores in lockstep -- a
  collective-free NEFF gets per-core launch skew of milliseconds.
- All weights are cast to fp16 AND pre-arranged into the exact SBUF tile
  layout on the HOST, so device DMAs are plain contiguous HWDGE transfers
  (~350GB/s observed; no SWDGE casting). Chain-critical small DMAs (AR
  bounces, x0) ride the scalar HWDGE ring so they never queue behind bulk
  weight DMA on the sync ring.
- Residual stream kept feature-major [128, 6, 32] fp16 (vector ops at packed
  rate, no fp16 staging copies); LayerNorm stats via PE ones-vector matmuls
  with 1/E folded into the ones constant; softmax skips max-subtraction
  (scores*scale is small); V's projection matmuls are emitted after the
  scores so the PE works under the softmax's vector/scalar ops.
- HAM keepalive: dummy fp32 matmuls pinned into each AllReduce gap keep the
  PE activity monitor busy so it stays at 2.4GHz instead of dropping to
  1.2GHz after >3.4us idle.
- fp16 matmuls with fp32 PSUM accumulation throughout; rel err ~1.8e-3.

Self-contained: shapes hardcoded, no sibling imports.
"""
import os
import sys
import types

import numpy as np

# If BASS_TRACE is set but the axon NTFF hook module is absent, bass_utils
# would crash importing antenv.axon_hooks. Provide a null hook so tracing
# degrades to a warning instead. (test.py installs the real hook first.)
try:
    from antenv import axon_hooks as _ah  # noqa: F401
except ImportError:
    try:
        import antenv as _antenv
        _mod = types.ModuleType("antenv.axon_hooks")
        _mod.get_axon_ntff_profile_hook = lambda: None
        _mod.set_axon_ntff_profile_hook = lambda h: None
        _antenv.axon_hooks = _mod
        sys.modules["antenv.axon_hooks"] = _mod
    except Exception:
        pass

import concourse.bass as bass
import concourse.bacc as bacc
import concourse.mybir as mybir
import concourse.tile as tile
from concourse import masks
from concourse.bass_utils import run_bass_kernel_spmd

F32 = mybir.dt.float32
F16 = mybir.dt.float16
AX = mybir.AxisListType
ALU = mybir.AluOpType
ACT_F = mybir.ActivationFunctionType

V, E, F, L, S, B, C = 30522, 768, 3072, 4, 512, 32, 2
NC = 8            # cores
KC = E // 128     # 6 contraction chunks of 128
MQ = (3 * E) // 128   # 18 output blocks for merged QKV
NSH = 8           # FFN shard ways (layers >= 1; layer 0's FFN is replicated
                  # so it needs no collective and can finish during the ~75us
                  # ncfw startup barrier)
FSH = F // NSH    # 384 ffn shard
MH = FSH // 128   # 3 output blocks for FFN1 shard
def _mh(l):
    return F // 128 if l < 2 else MH
SCALE = 1.0 / float(np.sqrt(E))
EPS = 1e-5

_CACHE = {}
LAST_RESULT = None  # BassKernelResults of the most recent run (for test.py)


def _declare(nc, use_bias, use_affine):
    h = {}
    h["x0T"] = nc.dram_tensor("x0T", [128, KC * B], F16, kind="ExternalInput")
    for l in range(L):
        h[f"wqkv{l}"] = nc.dram_tensor(f"wqkv{l}", [128, MQ * KC * 128], F16, kind="ExternalInput")
        h[f"wo{l}"] = nc.dram_tensor(f"wo{l}", [128, KC * KC * 128], F16, kind="ExternalInput")
        h[f"w1{l}"] = nc.dram_tensor(f"w1{l}", [128, KC * _mh(l) * 128], F16, kind="ExternalInput")
        h[f"w2{l}"] = nc.dram_tensor(f"w2{l}", [128, _mh(l) * KC * 128], F16, kind="ExternalInput")
        if use_bias:
            pass
        if use_bias:
            h[f"bqkv{l}"] = nc.dram_tensor(f"bqkv{l}", [128, MQ], F32, kind="ExternalInput")
            h[f"bo{l}"] = nc.dram_tensor(f"bo{l}", [128, KC], F32, kind="ExternalInput")
            h[f"bf1{l}"] = nc.dram_tensor(f"bf1{l}", [128, _mh(l)], F32, kind="ExternalInput")
            h[f"bf2{l}"] = nc.dram_tensor(f"bf2{l}", [128, KC], F32, kind="ExternalInput")
        if use_affine:
            h[f"g1{l}"] = nc.dram_tensor(f"g1{l}", [128, KC], F32, kind="ExternalInput")
            h[f"be1{l}"] = nc.dram_tensor(f"be1{l}", [128, KC], F32, kind="ExternalInput")
            h[f"g2{l}"] = nc.dram_tensor(f"g2{l}", [128, KC], F32, kind="ExternalInput")
            h[f"be2{l}"] = nc.dram_tensor(f"be2{l}", [128, KC], F32, kind="ExternalInput")
    h["wc"] = nc.dram_tensor("wc", [128, KC * C], F16, kind="ExternalInput")
    if use_bias:
        h["bc"] = nc.dram_tensor("bc", [B, C], F32, kind="ExternalInput")
    h["out"] = nc.dram_tensor("out", [B, C], F32, kind="ExternalOutput")
    return h


def _emit(tc, h, use_bias, use_affine):
    nc = tc.nc
    ctxs = []
    rx_waits = []  # (reduce BassInstruction, threshold) — waits attached post-schedule

    def pool(*a, **k):
        p = tc.alloc_tile_pool(*a, **k)
        ctxs.append(p)
        return p

    const = pool(name="const", bufs=1)
    wp = pool(name="wts", bufs=3)
    ab = pool(name="act", bufs=2)
    ps = pool(name="ps", bufs=2, space="PSUM")
    dr = pool(name="dram", bufs=2, space="DRAM")

    groups = [list(range(NC))]
    rx_sem = None

    wu_sb = const.tile([1, 16], F32)
    nc.vector.memset(wu_sb[:], 0.0)
    wu_i = dr.tile([16], F32, tag="wui")
    wu_o = dr.tile([16], F32, addr_space="Shared", tag="wuo")
    nc.scalar.dma_start(wu_i[:].rearrange("(o x) -> o x", o=1), wu_sb[:])
    nc.gpsimd.collective_compute(
        "AllReduce", ALU.add, replica_groups=groups,
        ins=[wu_i.opt()], outs=[wu_o.opt()],
    )

    # ---- constants
    ones_col = const.tile([128, 1], F32)
    nc.vector.memset(ones_col[:], 1.0 / E)
    ones_col16 = const.tile([128, 1], F16)
    nc.vector.memset(ones_col16[:], 1.0 / E)
    ones_row = const.tile([1, 128], F32)
    nc.vector.memset(ones_row[:], 1.0)
    eps_sb = const.tile([1, 1], F32)
    nc.vector.memset(eps_sb[:], EPS)
    ident128 = const.tile([128, 128], F16)
    masks.make_identity(nc, ident128[:])
    warm_src = const.tile([128, 512], F32)
    nc.vector.memset(warm_src[:], 1.0)

    # ---- x0 (host-combined tok+pos embedding, feature-major)
    xT = ab.tile([128, KC, B], F16, tag="xt")
    nc.scalar.dma_start(xT[:], h["x0T"].ap().rearrange("p (k b) -> p k b", k=KC))

    def load_vec(name):
        # [128, n] f32 per-feature column vectors (bias/affine)
        t = wp.tile([128, h[name].shape[1]], F32, tag=name[:3])
        nc.sync.dma_start(t[:], h[name].ap())
        return t

    def layernorm(yT, g=None, be=None):
        sq = ab.tile([128, KC, B], F32, tag="sq")
        nc.vector.tensor_tensor(sq[:], yT[:], yT[:], op=ALU.mult)
        s1 = ps.tile([1, B], F32, tag="st")
        s2 = ps.tile([1, B], F32, tag="st")
        for k in range(KC):
            nc.tensor.matmul(s1[:], ones_col16[:], yT[:, k, :], start=(k == 0), stop=(k == KC - 1))
        for k in range(KC):
            nc.tensor.matmul(s2[:], ones_col[:], sq[:, k, :], start=(k == 0), stop=(k == KC - 1))
        # s1 = mean, s2 = E[x^2] (ones_col holds 1/E)
        mean = ab.tile([1, B], F32, tag="mean")
        nc.vector.tensor_copy(mean[:], s1[:])
        msq = ab.tile([1, B], F32, tag="msq")
        nc.scalar.activation(msq[:], mean[:], ACT_F.Square)
        var = ab.tile([1, B], F32, tag="var")
        nc.vector.tensor_tensor(var[:], s2[:], msq[:], op=ALU.subtract)
        sd = ab.tile([1, B], F32, tag="sd")
        nc.scalar.activation(sd[:], var[:], ACT_F.Sqrt, bias=eps_sb[:])
        rstd = ab.tile([1, B], F32, tag="rstd")
        nc.vector.reciprocal(rstd[:], sd[:])
        mu_b = ps.tile([128, B], F32, tag="st")
        nc.tensor.matmul(mu_b[:], ones_row[:], mean[:], start=True, stop=True)
        rs_b = ps.tile([128, B], F32, tag="st")
        nc.tensor.matmul(rs_b[:], ones_row[:], rstd[:], start=True, stop=True)
        xn = ab.tile([128, KC, B], F16, tag="xn")
        tmp = ab.tile([128, KC, B], F16, tag="lntmp")
        mu_bb = mu_b[:].rearrange("p (o b) -> p o b", o=1).broadcast_to([128, KC, B])
        rs_bb = rs_b[:].rearrange("p (o b) -> p o b", o=1).broadcast_to([128, KC, B])
        nc.vector.tensor_tensor(tmp[:], yT[:], mu_bb, op=ALU.subtract)
        nc.vector.tensor_tensor(xn[:], tmp[:], rs_bb, op=ALU.mult)
        if g is not None:
            for k in range(KC):
                if be is not None:
                    nc.vector.tensor_scalar(
                        xn[:, k, :], xn[:, k, :], g[:, k:k + 1], be[:, k:k + 1],
                        ALU.mult, ALU.add)
                else:
                    nc.vector.tensor_scalar_mul(xn[:, k, :], xn[:, k, :], g[:, k:k + 1])
        elif be is not None:
            for k in range(KC):
                nc.vector.tensor_scalar_add(xn[:, k, :], xn[:, k, :], be[:, k:k + 1])
        return xn

    for l in range(L):
        # ---- weight loads: plain fp16 HWDGE streams, host-prearranged layout
        wqkv_t = wp.tile([128, MQ * KC, 128], F16, tag="wqkv", bufs=1)
        nc.sync.dma_start(wqkv_t[:], h[f"wqkv{l}"].ap().rearrange("p (x f) -> p x f", f=128))
        wo_t = wp.tile([128, KC * KC, 128], F16, tag="wo", bufs=1)
        nc.sync.dma_start(wo_t[:], h[f"wo{l}"].ap().rearrange("p (x f) -> p x f", f=128))
        mh = _mh(l)
        w1_t = wp.tile([128, KC * mh, 128], F16,
                       tag="w1big" if l < 2 else f"w1_{l}", bufs=1)
        nc.sync.dma_start(w1_t[:], h[f"w1{l}"].ap().rearrange("p (x f) -> p x f", f=128))
        w2_t = wp.tile([128, mh * KC, 128], F16,
                       tag="w2big" if l < 2 else f"w2_{l}", bufs=1)
        nc.sync.dma_start(w2_t[:], h[f"w2{l}"].ap().rearrange("p (x f) -> p x f", f=128))
        bqkv = load_vec(f"bqkv{l}") if use_bias else None
        bo = load_vec(f"bo{l}") if use_bias else None
        bf1 = load_vec(f"bf1{l}") if use_bias else None
        bf2 = load_vec(f"bf2{l}") if use_bias else None
        g1 = load_vec(f"g1{l}") if use_affine else None
        be1 = load_vec(f"be1{l}") if use_affine else None
        g2 = load_vec(f"g2{l}") if use_affine else None
        be2 = load_vec(f"be2{l}") if use_affine else None

        # ---- merged QKV^T, feature-major [128, 18, 32]
        qkv_ps = ps.tile([128, MQ, B], F32, tag="qkv", bufs=1)
        for m in range(2 * KC):
            for k in range(KC):
                nc.tensor.matmul(qkv_ps[:, m, :], wqkv_t[:, k * MQ + m, :], xT[:, k, :],
                                 start=(k == 0), stop=(k == KC - 1))
        qkvT = ab.tile([128, MQ, B], F16, tag="qkvT")
        if use_bias:
            for m in range(2 * KC):
                nc.vector.tensor_scalar_add(qkvT[:, m, :], qkv_ps[:, m, :], bqkv[:, m:m + 1])
        else:
            nc.vector.tensor_copy(qkvT[:, 0:2 * KC, :], qkv_ps[:, 0:2 * KC, :])

        # ---- scores + softmax (token-major [32, 32]); V's matmuls run on the
        # PE underneath the vector/scalar softmax work
        sc_ps = ps.tile([B, B], F32, tag="sc", bufs=1)
        for k in range(KC):
            nc.tensor.matmul(sc_ps[:], qkvT[:, k, :], qkvT[:, KC + k, :],
                             start=(k == 0), stop=(k == KC - 1))
        for m in range(2 * KC, MQ):
            for k in range(KC):
                nc.tensor.matmul(qkv_ps[:, m, :], wqkv_t[:, k * MQ + m, :], xT[:, k, :],
                                 start=(k == 0), stop=(k == KC - 1))
        if use_bias:
            for m in range(2 * KC, MQ):
                nc.vector.tensor_scalar_add(qkvT[:, m, :], qkv_ps[:, m, :], bqkv[:, m:m + 1])
        else:
            nc.vector.tensor_copy(qkvT[:, 2 * KC:MQ, :], qkv_ps[:, 2 * KC:MQ, :])
        attn = ab.tile([B, B], F32, tag="attn")
        rsum = ab.tile([B, 1], F32, tag="rsum")
        nc.scalar.activation(attn[:], sc_ps[:], ACT_F.Exp, scale=SCALE,
                             accum_out=rsum[:])
        rinv = ab.tile([B, 1], F32, tag="rinv")
        nc.vector.reciprocal(rinv[:], rsum[:])
        attn_n = ab.tile([B, B], F16, tag="attn_n")
        nc.vector.tensor_scalar_mul(attn_n[:], attn[:], rinv[:])
        attnT = ab.tile([B, B], F16, tag="attnT")
        nc.vector.transpose(attnT[:], attn_n[:])

        # ---- V token-major via PE transposes, then ao^T, then o^T = Wo^T ao^T
        v_ps = ps.tile([B, KC, 128], F16, tag="mm")
        for k in range(KC):
            nc.tensor.transpose(v_ps[:, k, :], qkvT[:, 2 * KC + k, :], ident128[:])
        v_sb = ab.tile([B, KC, 128], F16, tag="vsb")
        nc.vector.tensor_copy(v_sb[:], v_ps[:])
        ao_ps = ps.tile([128, KC, B], F32, tag="mm")
        for m in range(KC):
            nc.tensor.matmul(ao_ps[:, m, :], v_sb[:, m, :], attnT[:], start=True, stop=True)
        aoT = ab.tile([128, KC, B], F16, tag="aoT")
        nc.vector.tensor_copy(aoT[:], ao_ps[:])
        oT_ps = ps.tile([128, KC, B], F32, tag="mm")
        for m in range(KC):
            for k in range(KC):
                nc.tensor.matmul(oT_ps[:, m, :], wo_t[:, k * KC + m, :], aoT[:, k, :],
                                 start=(k == 0), stop=(k == KC - 1))

        # ---- residual + LN1
        y1 = ab.tile([128, KC, B], F16, tag="y1")
        y1_add = nc.vector.tensor_tensor(y1[:], xT[:], oT_ps[:], op=ALU.add)
        if use_bias:
            for k in range(KC):
                nc.vector.tensor_scalar_add(y1[:, k, :], y1[:, k, :], bo[:, k:k + 1])
        x1n = layernorm(y1, g1, be1)

        # ---- FFN1 shard: h^T = relu(W1_c^T x1n)  [128, 3, 32]
        hT_ps = ps.tile([128, mh, B], F32, tag="mm" if l >= 2 else "qkv", bufs=2 if l >= 2 else 1)
        for m in range(mh):
            for k in range(KC):
                nc.tensor.matmul(hT_ps[:, m, :], w1_t[:, k * mh + m, :], x1n[:, k, :],
                                 start=(k == 0), stop=(k == KC - 1))
        if use_bias:
            for m in range(mh):
                nc.vector.tensor_scalar_add(hT_ps[:, m, :], hT_ps[:, m, :], bf1[:, m:m + 1])
        hT = ab.tile([128, mh, B], F16, tag=f"hT{l}", bufs=1)
        nc.vector.tensor_scalar_max(hT[:], hT_ps[:], 0.0)

        # ---- FFN2: o2^T (full for layer 0, shard partial for layers >= 1)
        o2_ps = ps.tile([128, KC, B], F32, tag="mm")
        for m in range(KC):
            for t in range(mh):
                nc.tensor.matmul(o2_ps[:, m, :], w2_t[:, t * KC + m, :], hT[:, t, :],
                                 start=(t == 0), stop=(t == mh - 1))
        if l < 2:
            y2 = ab.tile([128, KC, B], F16, tag="y2")
            y2_add = nc.vector.tensor_tensor(y2[:], x1n[:], o2_ps[:], op=ALU.add)
            if l == 1:
                # ncfw keepalives: after its last op finishes, ncfw re-sleeps
                # and the next op pays a ~20us wake penalty (AR-2 measured
                # 29us after an idle gap vs ~10us back-to-back). Two thrown-
                # away collectives, semaphore-pinned into layer 1, keep it
                # busy until the first real AllReduce arrives.
                for ki, anchor in ((0, y1_add), (1, y2_add)):
                    ka_i = dr.tile([16], F32, tag=f"ka{ki}i")
                    ka_o = dr.tile([16], F32, addr_space="Shared", tag=f"ka{ki}o")
                    kd = nc.scalar.dma_start(
                        ka_i[:].rearrange("(o x) -> o x", o=1), wu_sb[:])
                    tile.add_dep_helper(kd.ins, anchor.ins, sync=True,
                                        reason="ncfw keepalive pacing")
                    nc.gpsimd.collective_compute(
                        "AllReduce", ALU.add, replica_groups=groups,
                        ins=[ka_i.opt()], outs=[ka_o.opt()],
                    )
            if use_bias:
                for k in range(KC):
                    nc.vector.tensor_scalar_add(y2[:, k, :], y2[:, k, :], bf2[:, k:k + 1])
            xT = layernorm(y2, g2, be2)
            continue
        o2s = ab.tile([128, KC, B], F16, tag=f"o2s{l}", bufs=1)
        nc.vector.tensor_copy(o2s[:], o2_ps[:])

        o2g = ab.tile([128, KC * B], F16, tag="o2g")
        if True:
            ar_i = dr.tile([128 * KC * B], F16, tag="ari")
            ar_o = dr.tile([128 * KC * B], F16, addr_space="Shared", tag="aro")
            nc.scalar.dma_start(
                ar_i[:].rearrange("(p k b) -> p k b", p=128, k=KC), o2s[:])
            cc = nc.gpsimd.collective_compute(
                "AllReduce", ALU.add, replica_groups=groups,
                ins=[ar_i.opt()], outs=[ar_o.opt()],
            )
            nc.scalar.dma_start(
                o2g[:].rearrange("p (k b) -> p k b", k=KC),
                ar_o[:].rearrange("(p k b) -> p k b", p=128, k=KC))
            # HAM keepalive: the AR leaves the PE idle past the ~3.4us HAM
            # window, dropping it to 1.2GHz for the next layer. Dummy fp32
            # matmuls (~0.85us each) pinned behind the AR trigger keep the
            # activity monitor busy; results go to a scratch PSUM bank.
            warm_ps = ps.tile([1, 512], F32, tag="sc", bufs=1)
            for _ in range(8):
                d = nc.tensor.matmul(warm_ps[:], ones_col[:], warm_src[:],
                                     start=True, stop=True)
                tile.add_dep_helper(d.ins, cc.ins, sync=False,
                                    reason="HAM keepalive pinned in AR gap")

        # ---- residual + LN2
        y2 = ab.tile([128, KC, B], F16, tag="y2")
        nc.vector.tensor_tensor(
            y2[:], x1n[:], o2g[:].rearrange("p (k b) -> p k b", k=KC), op=ALU.add)
        if use_bias:
            for k in range(KC):
                nc.vector.tensor_scalar_add(y2[:, k, :], y2[:, k, :], bf2[:, k:k + 1])
        xT = layernorm(y2, g2, be2)

    # ---- classifier
    wc_sb = wp.tile([128, KC, C], F16, tag="wc")
    nc.sync.dma_start(wc_sb[:], h["wc"].ap().rearrange("p (k n) -> p k n", k=KC))
    lg_ps = ps.tile([B, C], F32, tag="sc", bufs=1)
    for k in range(KC):
        nc.tensor.matmul(lg_ps[:], xT[:, k, :], wc_sb[:, k, :], start=(k == 0), stop=(k == KC - 1))
    lg_sb = ab.tile([B, C], F32, tag="lgs")
    if use_bias:
        bc_sb = wp.tile([B, C], F32, tag="bcs")
        nc.sync.dma_start(bc_sb[:], h["bc"].ap())
        nc.vector.tensor_tensor(lg_sb[:], lg_ps[:], bc_sb[:], op=ALU.add)
    else:
        nc.vector.tensor_copy(lg_sb[:], lg_ps[:])
    nc.scalar.dma_start(h["out"].ap(), lg_sb[:])

    for p in reversed(ctxs):
        p.release()
    return rx_waits, rx_sem


def build(use_bias, use_affine):
    key = (use_bias, use_affine)
    if key in _CACHE:
        return _CACHE[key]
    nc = bacc.Bacc("TRN2", target_bir_lowering=False, debug=False, num_devices=NC)
    h = _declare(nc, use_bias, use_affine)
    with tile.TileContext(nc) as tc:
        rx_waits, rx_sem = _emit(tc, h, use_bias, use_affine)
    # Attach the true cross-core arrival waits AFTER scheduling: the Tile
    # scheduling sim cannot model remote semaphore increments and would
    # report a false deadlock. The wait condition rides on the reduce
    # instruction itself, so it is enforced wherever it was scheduled.
    for red, thr in rx_waits:
        red.wait_op(rx_sem, thr, "sem-ge")
    nc.compile()
    _CACHE[key] = (nc, h)
    return nc, h


def _prep_w(w, nblk):
    # [rows, cols] fp32 -> [128, (rows/128)*(cols/128)*128] fp16, laid out so
    # SBUF tile slice [:, k*nblk+m, :] is the [128,128] block W[128k:, 128m:].
    rows, cols = w.shape
    k = rows // 128
    assert cols == nblk * 128
    return np.ascontiguousarray(
        w.reshape(k, 128, nblk, 128).transpose(1, 0, 2, 3).reshape(128, -1)
        .astype(np.float16))


def _prep_vec(v):
    # [n] -> [128, n/128] f32 feature-major column layout
    n = v.shape[0]
    return np.ascontiguousarray(v.reshape(n // 128, 128).T.astype(np.float32))


def make_in_maps(inputs, use_bias, use_affine):
    inp = {k: np.asarray(v, dtype=np.float32)
           if np.asarray(v).dtype not in (np.int32, np.int64) else np.asarray(v)
           for k, v in inputs.items()}
    ids = np.asarray(inputs["input_ids"])[0]
    x0 = inp["tok_emb"][ids] + inp["pos_emb"][0][None, :]       # [32, 768]
    x0T = np.ascontiguousarray(
        x0.T.reshape(KC, 128, B).transpose(1, 0, 2).reshape(128, KC * B)
        .astype(np.float16))

    # replicated tensors (same arrays shared across cores)
    rep = {"x0T": x0T}
    for l in range(L):
        wqkv = np.concatenate([inp["Wq"][l], inp["Wk"][l], inp["Wv"][l]], axis=1)
        rep[f"wqkv{l}"] = _prep_w(wqkv, MQ)
        rep[f"wo{l}"] = _prep_w(inp["Wo"][l], KC)
        if use_bias:
            bqkv = np.concatenate([inp["bq"][l], inp["bk"][l], inp["bv"][l]])
            rep[f"bqkv{l}"] = _prep_vec(bqkv)
            rep[f"bo{l}"] = _prep_vec(inp["bo"][l])
            rep[f"bf2{l}"] = _prep_vec(inp["bf2"][l])
        if use_affine:
            rep[f"g1{l}"] = _prep_vec(inp["g1"][l])
            rep[f"be1{l}"] = _prep_vec(inp["beta1"][l])
            rep[f"g2{l}"] = _prep_vec(inp["g2"][l])
            rep[f"be2{l}"] = _prep_vec(inp["beta2"][l])
    rep["wc"] = np.ascontiguousarray(
        inp["Wc"].reshape(KC, 128, C).transpose(1, 0, 2).reshape(128, KC * C)
        .astype(np.float16))
    if use_bias:
        rep["bc"] = np.ascontiguousarray(
            np.broadcast_to(inp["bc"][None, :], (B, C)).astype(np.float32))

    rep_w1f = [_prep_w(inp["W1"][l], F // 128) for l in range(2)]
    rep_w2f = [_prep_w(inp["W2"][l], KC) for l in range(2)]
    rep_bf1f = [_prep_vec(inp["bf1"][l]) for l in range(2)] if use_bias else None
    in_maps = []
    for c in range(NC):
        s = c & (NSH - 1)  # cores c and c^4 carry the same FFN shard
        m = dict(rep)
        for l in range(L):
            if l < 2:
                m[f"w1{l}"] = rep_w1f[l]
                m[f"w2{l}"] = rep_w2f[l]
                if use_bias:
                    m[f"bf1{l}"] = rep_bf1f[l]
                continue
            m[f"w1{l}"] = _prep_w(
                np.ascontiguousarray(inp["W1"][l][:, FSH * s:FSH * (s + 1)]), MH)
            m[f"w2{l}"] = _prep_w(
                np.ascontiguousarray(inp["W2"][l][FSH * s:FSH * (s + 1), :]), KC)
            if use_bias:
                m[f"bf1{l}"] = _prep_vec(inp["bf1"][l][FSH * s:FSH * (s + 1)])
        in_maps.append(m)
    return in_maps


def _flags(inputs):
    z = lambda *names: all(not np.any(np.asarray(inputs[n])) for n in names)
    use_bias = not z("bq", "bk", "bv", "bo", "bf1", "bf2", "bc")
    use_affine = not (
        z("beta1", "beta2")
        and np.all(np.asarray(inputs["g1"]) == 1.0)
        and np.all(np.asarray(inputs["g2"]) == 1.0)
    )
    return use_bias, use_affine


def kernel(**inputs) -> np.ndarray:
    global LAST_RESULT
    use_bias, use_affine = _flags(inputs)
    nc, h = build(use_bias, use_affine)
    in_maps = make_in_maps(inputs, use_bias, use_affine)
    res = run_bass_kernel_spmd(nc, in_maps, core_ids=list(range(NC)))
    LAST_RESULT = res
    return np.asarray(res.results[0]["out"])
